# revision 22
# baseline (speedup 1.0000x reference)
# GAT (graph attention) layer on 8 Trainium2 NeuronCores — v2.
#
# Target-sharded edges (cores own 6272-aligned target ranges).  Per core:
#   Phase T: gather-table rows [proj(128)|ssrc_hi|strg_hi|ssrc_lo|strg_lo]
#     built from HOST-pretransposed, host-split bf16 hi/lo xT (no on-device
#     transposes or conversions); fp32-accurate scores via 3 hi/lo matmuls.
#     s_trg hi/lo for the core's own targets (stc) from xlocT, kept in SBUF.
#   Phase E: per 128-target window, two ucode dma_gathers fetch [proj|s] rows
#     by src; s_trg per edge via one-hot matmul with a host-streamed
#     transposed one-hot (selt); z -> leaky -> exp(z-24); one matmul per edge
#     tile aggregates [weighted-proj | denom] with targets on partitions
#     (PSUM accumulate, no transposes); per-window results stay in SBUF.
#   Collective: AllReduce(max) of one scalar (global score max M) reproduces
#     the reference's exp(e - e.max()) + 1e-16 epsilon numerics exactly.
#   Phase F: out = elu(W/(D + 1e-16*exp(M-24)) + x + bias); x+bias is
#     precomputed during phase E, the rest runs as 4 DVE/Act-pipelined chunks.
import sys

import numpy as np

sys.path.insert(0, "/opt/trn_rl_repo")

import ml_dtypes  # noqa: E402

import concourse.bass as bass  # noqa: E402,F401
import concourse.bass_isa as bass_isa  # noqa: E402
import concourse.mybir as mybir  # noqa: E402
import concourse.tile as tile  # noqa: E402
from concourse import bacc  # noqa: E402

P = 128
NH, FOUT = 4, 32
NHF = NH * FOUT  # 128
FIN = 128
ROW = 256  # bf16 elems per table row (512B); 144 used
WEX = NHF + NH  # 132: [weighted proj | ex]
LEAKY = 0.2
SHIFT = 24.0
N_NODES = 50000
N_CORES = 8
NPC = 6272  # 49 * 128, per-core padded target count
NW = 49
NPAD = 50176  # 98 * 512 = 49 * 1024, padded node count
HALF = 25088  # table split (A: [0, 25088), B: [25088, 50176))
TABR = HALF + 1  # +1 pad row (idx 25088) holding -1e4
PADV = -1e4
F32 = mybir.dt.float32
BF16 = mybir.dt.bfloat16
I16 = mybir.dt.int16
AX = mybir.AxisListType
OP = mybir.AluOpType
ACT = mybir.ActivationFunctionType
BF = ml_dtypes.bfloat16


def _wrap16(flat):
    """[..., L] -> dma_gather layout [..., 128, L//16] (16-wrap, replicated)."""
    L = flat.shape[-1]
    w = flat.reshape(flat.shape[:-1] + (L // 16, 16))
    w = np.swapaxes(w, -1, -2)
    return np.tile(w, (1, 1, 8, 1)).reshape(flat.shape[:-1] + (P, L // 16))


def _prepare_edges(edge_index):
    src = np.ascontiguousarray(edge_index[0]).astype(np.int64)
    trg = np.ascontiguousarray(edge_index[1]).astype(np.int64)
    E = src.shape[0]
    wglob = (trg // NPC) * NW + (trg % NPC) // P  # 0..391
    isb = (src >= HALF).astype(np.int64)
    order = np.argsort(wglob * 2 + isb, kind="stable")
    src_s, trg_s, wg_s, isb_s = src[order], trg[order], wglob[order], isb[order]
    nwin = N_CORES * NW
    cnt = np.bincount(wg_s * 2 + isb_s, minlength=2 * nwin)
    t_a = max(1, int(np.ceil(cnt[0::2].max() / P)))
    t_b = max(1, int(np.ceil(cnt[1::2].max() / P)))
    t_eff = t_a + t_b
    gkey = wg_s * 2 + isb_s
    gstart = np.concatenate([[0], np.cumsum(np.bincount(gkey, minlength=2 * nwin))])[:-1]
    jj = np.arange(E) - gstart[gkey]
    t_loc = jj // P
    p_idx = jj % P
    t_idx = np.where(isb_s == 1, t_a + t_loc, t_loc)
    c = wg_s // NW
    wloc = wg_s % NW
    rel = (trg_s % NPC) - wloc * P  # 0..127

    idx_a = np.full((N_CORES, NW, t_a * P), HALF, np.int16)  # pad row
    idx_b = np.full((N_CORES, NW, t_b * P), HALF, np.int16)
    ma = isb_s == 0
    idx_a[c[ma], wloc[ma], t_loc[ma] * P + p_idx[ma]] = src_s[ma].astype(np.int16)
    mb = isb_s == 1
    idx_b[c[mb], wloc[mb], t_loc[mb] * P + p_idx[mb]] = (src_s[mb] - HALF).astype(np.int16)

    rel_arr = np.full((N_CORES, NW * P, t_eff), -1.0, np.float32)
    rel_arr[c, wloc * P + p_idx, t_idx] = rel
    selt = np.zeros((N_CORES, NW * P, t_eff * P), BF)
    selt[c, wloc * P + rel, t_idx * P + p_idx] = 1.0

    ia = _wrap16(idx_a)
    ib = _wrap16(idx_b)
    gidx = np.concatenate([ia, ib], axis=-1).reshape(N_CORES, NW * P, t_eff * 8)
    return t_a, t_b, gidx, rel_arr.astype(BF), selt


def build_bass(t_a, t_b, sim_no_cc=False):
    t_eff = t_a + t_b
    nc = bacc.Bacc("TRN2", target_bir_lowering=False, debug=False,
                   num_devices=N_CORES)

    xTh = nc.dram_tensor("xTh", [P, NPAD], BF16, kind="ExternalInput")
    xTl = nc.dram_tensor("xTl", [P, NPAD], BF16, kind="ExternalInput")
    xlTh = nc.dram_tensor("xlTh", [P, NPC], BF16, kind="ExternalInput")
    xlTl = nc.dram_tensor("xlTl", [P, NPC], BF16, kind="ExternalInput")
    xloc = nc.dram_tensor("xloc", [NPC, FIN], F32, kind="ExternalInput")
    wcatb = nc.dram_tensor("wcatb", [P, 144], BF16, kind="ExternalInput")
    bias_in = nc.dram_tensor("bias", [1, NHF], F32, kind="ExternalInput")
    gidx_in = nc.dram_tensor("gidx", [NW * P, t_eff * 8], I16,
                             kind="ExternalInput")
    rels_in = nc.dram_tensor("rels", [NW * P, t_eff], BF16,
                             kind="ExternalInput")
    selt_in = nc.dram_tensor("selt", [NW * P, t_eff * P], BF16,
                             kind="ExternalInput")
    out = nc.dram_tensor("out", [NPC, NHF], F32, kind="ExternalOutput")

    tab_a = nc.dram_tensor("tab_a", [TABR, ROW], BF16)
    tab_b = nc.dram_tensor("tab_b", [TABR, ROW], BF16)

    with tile.TileContext(nc) as tc:
        with tc.tile_pool(name="const", bufs=1) as const:
            # --- consts and preloads ---
            wc = const.tile([P, 144], BF16)
            nc.sync.dma_start(wc[:], wcatb[:])
            b1 = const.tile([1, NHF], F32)
            nc.sync.dma_start(b1[:], bias_in[:])
            sbias = const.tile([P, NHF], F32)
            nc.gpsimd.partition_broadcast(sbias[:], b1[:])
            c_i32 = const.tile([P, P], mybir.dt.int32)
            nc.gpsimd.iota(c_i32[:], pattern=[[1, P]], base=0,
                           channel_multiplier=0)
            c_bf = const.tile([P, P], BF16)
            nc.vector.tensor_copy(c_bf[:], c_i32[:])
            bias_m24 = const.tile([P, 1], F32)
            nc.gpsimd.memset(bias_m24[:], -SHIFT)
            bias0 = const.tile([P, 1], F32)
            nc.gpsimd.memset(bias0[:], 0.0)
            padrow = const.tile([1, ROW], BF16)
            nc.gpsimd.memset(padrow[:], PADV)
            nc.sync.dma_start(tab_a[HALF:HALF + 1, :], padrow[:])
            nc.sync.dma_start(tab_b[HALF:HALF + 1, :], padrow[:])
            zmax = const.tile([P, t_eff * NH], F32)
            nc.gpsimd.memset(zmax[:], -1e30)
            stc = const.tile([P, NW * 8], BF16)  # [strg_hi(4)|strg_lo(4)]
            stc3 = stc[:].rearrange("p (w c) -> p w c", c=8)
            sWD = const.tile([P, NW * WEX], F32)  # [W(128)|D(4)] per window
            sWD3 = sWD[:].rearrange("p (w c) -> p w c", c=WEX)
            gidx_all = const.tile([P, NW * t_eff * 8], I16)
            nc.sync.dma_start(
                gidx_all[:].rearrange("p (w f) -> p w f", w=NW),
                gidx_in[:].rearrange("(w p) f -> p w f", p=P))
            gidx3 = gidx_all[:].rearrange("p (w f) -> p w f", w=NW)
            rel_all = const.tile([P, NW * t_eff], BF16)
            nc.sync.dma_start(
                rel_all[:].rearrange("p (w f) -> p w f", w=NW),
                rels_in[:].rearrange("(w p) f -> p w f", p=P))
            rel3 = rel_all[:].rearrange("p (w f) -> p w f", w=NW)

            # --- phase T: build gather tables from host-split xT hi/lo ---
            # ps cols 0:128 proj, 128:136 full fp32 scores [ssrc|strg] via
            # hi*W[0:144] + hi*wsa_lo + lo*wsa_hi into the 128:136 window.
            with tc.tile_pool(name="sbT", bufs=4) as sbT, \
                 tc.tile_pool(name="psT", bufs=2, space="PSUM") as psT:
                for sb_i in range(NPAD // 1024):
                    r0 = sb_i * 1024
                    xh = sbT.tile([P, 1024], BF16, tag="xh")
                    nc.sync.dma_start(xh[:], xTh[:, r0:r0 + 1024])
                    xl = sbT.tile([P, 1024], BF16, tag="xl")
                    nc.sync.dma_start(xl[:], xTl[:, r0:r0 + 1024])
                    tabt = sbT.tile([P, 8 * ROW], BF16, tag="tabt")
                    tb3 = tabt[:].rearrange("p (k f) -> p k f", f=ROW)
                    nc.gpsimd.memset(tb3[:, :, 144:256], 0.0)
                    for pair in range(4):
                        ps_p = psT.tile([P, 2 * 128], F32, tag="ps_p")
                        pp3 = ps_p[:].rearrange("p (i f) -> p i f", f=128)
                        ps_s = psT.tile([P, 2 * 8], F32, tag="ps_s")
                        ss3 = ps_s[:].rearrange("p (i f) -> p i f", f=8)
                        for i in range(2):
                            o = (pair * 2 + i) * 128
                            lhi = xh[:, o:o + 128]
                            llo = xl[:, o:o + 128]
                            nc.tensor.matmul(pp3[:, i, :], lhsT=lhi,
                                             rhs=wc[:, 0:128], start=True,
                                             stop=True)
                            nc.tensor.matmul(ss3[:, i, :], lhsT=lhi,
                                             rhs=wc[:, 128:136], start=True,
                                             stop=False)
                            nc.tensor.matmul(ss3[:, i, :], lhsT=lhi,
                                             rhs=wc[:, 136:144], start=False,
                                             stop=False)
                            nc.tensor.matmul(ss3[:, i, :], lhsT=llo,
                                             rhs=wc[:, 128:136], start=False,
                                             stop=True)
                        k = pair * 2
                        nc.scalar.copy(tb3[:, k:k + 2, 0:128], pp3[:])
                        # s slots: [128:136 hi | 136:144 lo], both [ssrc|strg]
                        nc.vector.tensor_copy(tb3[:, k:k + 2, 128:136], ss3)
                        slo = sbT.tile([P, 16], F32, tag="slo")
                        sl3 = slo[:].rearrange("p (i f) -> p i f", f=8)
                        nc.vector.tensor_tensor(sl3, ss3,
                                                tb3[:, k:k + 2, 128:136],
                                                OP.subtract)
                        nc.vector.tensor_copy(tb3[:, k:k + 2, 136:144], sl3)
                    if r0 + 1024 <= HALF:
                        nc.sync.dma_start(
                            tab_a[r0:r0 + 1024, :].rearrange(
                                "(k p) f -> p k f", p=P),
                            tb3[:, :, :])
                    elif r0 >= HALF:
                        nc.sync.dma_start(
                            tab_b[r0 - HALF:r0 - HALF + 1024, :].rearrange(
                                "(k p) f -> p k f", p=P),
                            tb3[:, :, :])
                    else:
                        nc.sync.dma_start(
                            tab_a[r0:HALF, :].rearrange(
                                "(k p) f -> p k f", p=P),
                            tb3[:, 0:4, :])
                        nc.sync.dma_start(
                            tab_b[0:r0 + 1024 - HALF, :].rearrange(
                                "(k p) f -> p k f", p=P),
                            tb3[:, 4:8, :])

                # --- stc: s_trg hi/lo for local targets from xlocT hi/lo ---
                for ci in range(7):
                    c0 = ci * 1024
                    clen = min(1024, NPC - c0)
                    xh = sbT.tile([P, 1024], BF16, tag="xh")
                    nc.sync.dma_start(xh[:, 0:clen], xlTh[:, c0:c0 + clen])
                    xl = sbT.tile([P, 1024], BF16, tag="xl")
                    nc.sync.dma_start(xl[:, 0:clen], xlTl[:, c0:c0 + clen])
                    for pair in range(max(1, clen // 256)):
                        nt2 = min(2, clen // 128 - pair * 2)
                        ps_c = psT.tile([P, 2 * 4], F32, tag="ps_c")
                        cc3 = ps_c[:].rearrange("p (i f) -> p i f", f=4)
                        for i in range(nt2):
                            o = (pair * 2 + i) * 128
                            lhi = xh[:, o:o + 128]
                            llo = xl[:, o:o + 128]
                            nc.tensor.matmul(cc3[:, i, :], lhsT=lhi,
                                             rhs=wc[:, 132:136], start=True,
                                             stop=False)
                            nc.tensor.matmul(cc3[:, i, :], lhsT=lhi,
                                             rhs=wc[:, 140:144], start=False,
                                             stop=False)
                            nc.tensor.matmul(cc3[:, i, :], lhsT=llo,
                                             rhs=wc[:, 132:136], start=False,
                                             stop=True)
                        w0 = c0 // P + pair * 2
                        nc.vector.tensor_copy(stc3[:, w0:w0 + nt2, 0:4],
                                              cc3[:, 0:nt2, :])
                        clo = sbT.tile([P, 8], F32, tag="clo")
                        cl3 = clo[:].rearrange("p (i f) -> p i f", f=4)
                        nc.vector.tensor_tensor(cl3[:, 0:nt2, :],
                                                cc3[:, 0:nt2, :],
                                                stc3[:, w0:w0 + nt2, 0:4],
                                                OP.subtract)
                        nc.vector.tensor_copy(stc3[:, w0:w0 + nt2, 4:8],
                                              cl3[:, 0:nt2, :])

            # --- phase E ---
            with tc.tile_pool(name="sbE", bufs=3) as sbE, \
                 tc.tile_pool(name="sbg", bufs=3) as sbg, \
                 tc.tile_pool(name="sbga", bufs=3) as sbga, \
                 tc.tile_pool(name="psE", bufs=3, space="PSUM") as psE:
                for w in range(NW):
                    gath = sbga.tile([P, t_eff * ROW], BF16, tag="gath")
                    g3 = gath[:].rearrange("p (t c) -> p t c", c=ROW)
                    nc.gpsimd.dma_gather(
                        out_ap=g3[:, 0:t_a, :], in_ap=tab_a[:],
                        idxs_ap=gidx3[:, w, 0:t_a * 8], num_idxs=t_a * P,
                        num_idxs_reg=t_a * P, elem_size=ROW,
                        single_packet=False)
                    nc.gpsimd.dma_gather(
                        out_ap=g3[:, t_a:t_eff, :], in_ap=tab_b[:],
                        idxs_ap=gidx3[:, w, t_a * 8:t_eff * 8],
                        num_idxs=t_b * P, num_idxs_reg=t_b * P, elem_size=ROW,
                        single_packet=False)
                    seltt = sbg.tile([P, t_eff * P], BF16, tag="selt")
                    nc.sync.dma_start(seltt[:],
                                      selt_in[w * P:(w + 1) * P, :])
                    selt_ap = seltt[:]

                    # s_trg per edge via one-hot matmul
                    ps_st = psE.tile([P, t_eff * 8], F32, tag="ps_st")
                    st3 = ps_st[:].rearrange("p (t c) -> p t c", c=8)
                    for t in range(t_eff):
                        nc.tensor.matmul(st3[:, t, :],
                                         lhsT=selt_ap[:, t * P:(t + 1) * P],
                                         rhs=stc3[:, w, :], start=True,
                                         stop=True)

                    # z = (ssrc_hi+strg_hi) + (ssrc_lo+strg_lo)
                    zs8 = sbE.tile([P, t_eff * 8], F32, tag="zs8")
                    z83 = zs8[:].rearrange("p (t h f) -> p t h f", h=2, f=4)
                    gsv = g3[:, :, 128:144].rearrange(
                        "p t (h g f) -> p t h g f", h=2, g=2)
                    nc.vector.tensor_tensor(
                        z83, gsv[:, :, :, 0, :],
                        st3.rearrange("p t (h f) -> p t h f", h=2), OP.add)
                    z = sbE.tile([P, t_eff * NH], F32, tag="z")
                    z3 = z[:].rearrange("p (t c) -> p t c", c=NH)
                    nc.vector.tensor_tensor(z3, z83[:, :, 0, :],
                                            z83[:, :, 1, :], OP.add)
                    nc.vector.tensor_tensor(zmax[:], zmax[:], z[:], OP.max)
                    # exp(lrelu(z)-24) == max(exp(0.2z-24), exp(z-24))
                    ea = sbE.tile([P, t_eff * NH], F32, tag="ea")
                    nc.scalar.activation(ea[:], z[:], ACT.Exp,
                                         bias=bias_m24[:], scale=LEAKY)
                    eb = sbE.tile([P, t_eff * NH], F32, tag="eb")
                    nc.scalar.activation(eb[:], z[:], ACT.Exp,
                                         bias=bias_m24[:])
                    wx = sbg.tile([P, t_eff * WEX], BF16, tag="wx")
                    wx3 = wx[:].rearrange("p (t c) -> p t c", c=WEX)
                    nc.vector.tensor_tensor(
                        wx3[:, :, 128:132],
                        ea[:].rearrange("p (t c) -> p t c", c=NH),
                        eb[:].rearrange("p (t c) -> p t c", c=NH), OP.max)

                    selb = sbg.tile([P, t_eff * P], BF16, tag="sel")
                    nc.vector.tensor_tensor(
                        selb[:].rearrange("p (t q) -> p t q", q=P),
                        rel3[:, w, :, None].to_broadcast([P, t_eff, P]),
                        c_bf[:, None, :].to_broadcast([P, t_eff, P]),
                        OP.is_equal)
                    sel = selb[:]
                    nc.vector.tensor_tensor(
                        wx3[:, :, 0:128].rearrange("p t (h f) -> p t h f",
                                                   f=FOUT),
                        g3[:, :, 0:128].rearrange("p t (h f) -> p t h f",
                                                  f=FOUT),
                        wx3[:, :, 128:132][:, :, :, None].to_broadcast(
                            [P, t_eff, NH, FOUT]),
                        OP.mult)

                    ps_o = psE.tile([P, WEX], F32, tag="ps_o")
                    for t in range(t_eff):
                        nc.tensor.matmul(ps_o[:],
                                         lhsT=sel[:, t * P:(t + 1) * P],
                                         rhs=wx3[:, t, :], start=(t == 0),
                                         stop=(t == t_eff - 1))
                    nc.scalar.copy(sWD3[:, w, :], ps_o[:])

                # --- global max + epsilon ---
                zm1 = sbE.tile([P, 1], F32, tag="zm1")
                nc.vector.tensor_reduce(zm1[:], zmax[:], axis=AX.X, op=OP.max)
                zma = sbE.tile([P, 1], F32, tag="zma")
                nc.gpsimd.partition_all_reduce(zma[:], zm1[:], channels=P,
                                               reduce_op=bass_isa.ReduceOp.max)
                with tc.tile_pool(name="dram", bufs=1, space="DRAM") as dram:
                    cc_in = dram.tile([1, 1], F32)
                    cc_out = dram.tile([1, 1], F32)
                    nc.sync.dma_start(cc_in[:], zma[0:1, :])
                    if sim_no_cc:
                        nc.sync.dma_start(cc_out[:], cc_in[:])
                    else:
                        nc.gpsimd.collective_compute(
                            "AllReduce", OP.max,
                            replica_groups=[list(range(N_CORES))],
                            ins=[cc_in.opt()], outs=[cc_out.opt()])
                    zg = sbE.tile([1, 1], F32, tag="zg")
                    nc.sync.dma_start(zg[:], cc_out[:])
                eg = sbE.tile([1, 1], F32, tag="eg")
                nc.vector.tensor_scalar_mul(eg[:], zg[:], LEAKY)
                nc.vector.tensor_tensor(eg[:], eg[:], zg[:], OP.max)
                ce = sbE.tile([1, 1], F32, tag="ce")
                nc.scalar.activation(ce[:], eg[:], ACT.Exp,
                                     bias=bias_m24[:1])
                nc.vector.tensor_scalar_mul(ce[:], ce[:], 1e-16)
                ceps = const.tile([P, 1], F32)
                nc.gpsimd.partition_broadcast(ceps[:], ce[:])
            # E pools closed here; phase F gets the freed SBUF
            with tc.tile_pool(name="sbFo", bufs=1) as sbFo:
                # --- phase F (xpb precomputed; 4 pipelined chunks) ---
                sbF = sbFo
                xw = sbF.tile([P, NW * NHF], F32, tag="xw")
                x3 = xw[:].rearrange("p (k f) -> p k f", f=NHF)
                nc.sync.dma_start(
                    x3, xloc[:].rearrange("(k p) f -> p k f", p=P))
                # x + bias does not depend on the collective; scheduled early
                nc.vector.tensor_tensor(
                    x3, x3, sbias[:, None, :].to_broadcast([P, NW, NHF]),
                    OP.add)
                den = sbF.tile([P, NW * NH], F32, tag="den")
                d3 = den[:].rearrange("p (k c) -> p k c", c=NH)
                nc.vector.tensor_tensor(
                    d3, sWD3[:, :, 128:132],
                    ceps[:, 0:1, None].to_broadcast([P, NW, NH]), OP.add)
                rec = sbF.tile([P, NW * NH], F32, tag="rec")
                nc.vector.reciprocal(rec[:], den[:])
                r3 = rec[:].rearrange("p (k c) -> p k c", c=NH)
                o1 = sbF.tile([P, NW * NHF], F32, tag="o1")
                o14 = o1[:].rearrange("p (k h f) -> p k h f", k=NW, h=NH)
                ee = sbF.tile([P, NW * NHF], F32, tag="ee")
                pos = sbF.tile([P, NW * NHF], F32, tag="pos")
                bounds = [0, 13, 25, 37, NW]
                for ci in range(4):
                    a, b = bounds[ci], bounds[ci + 1]
                    sl = slice(a * NHF, b * NHF)
                    nc.vector.tensor_tensor(
                        o14[:, a:b],
                        sWD3[:, a:b, 0:128].rearrange("p k (h f) -> p k h f",
                                                      f=FOUT),
                        r3[:, a:b, :, None].to_broadcast(
                            [P, b - a, NH, FOUT]),
                        OP.mult)
                    nc.vector.tensor_tensor(o1[:, sl], o1[:, sl], xw[:, sl],
                                            OP.add)
                    nc.scalar.activation(ee[:, sl], o1[:, sl], ACT.Exp,
                                         bias=bias0[:])
                    nc.scalar.activation(pos[:, sl], o1[:, sl], ACT.Relu,
                                         bias=bias0[:])
                    nc.vector.tensor_scalar(ee[:, sl], ee[:, sl], -1.0, 0.0,
                                            OP.add, OP.min)
                    nc.vector.tensor_tensor(ee[:, sl], ee[:, sl], pos[:, sl],
                                            OP.add)
                    nc.sync.dma_start(
                        out[a * P:b * P, :].rearrange("(k p) f -> p k f",
                                                      p=P),
                        ee[:].rearrange("p (k f) -> p k f", f=NHF)[:, a:b, :])

    nc.compile()
    return nc


def _make_inputs(x, edge_index, w_mat, a_src, a_trg, bias):
    t_a, t_b, gidx, rel_arr, selt = _prepare_edges(edge_index)
    x = np.ascontiguousarray(x, dtype=np.float32)
    xpad = np.zeros((NPAD, FIN), np.float32)
    xpad[:N_NODES] = x
    xT = np.ascontiguousarray(xpad.T)  # [128, 50176] f32
    xTh = xT.astype(BF)
    xTl = (xT - xTh.astype(np.float32)).astype(BF)

    asrc_m = np.zeros((NHF, NH), np.float32)
    atrg_m = np.zeros((NHF, NH), np.float32)
    for h in range(NH):
        asrc_m[h * FOUT:(h + 1) * FOUT, h] = a_src[h]
        atrg_m[h * FOUT:(h + 1) * FOUT, h] = a_trg[h]
    wsa = np.concatenate([w_mat @ asrc_m, w_mat @ atrg_m], axis=1)  # [128, 8]
    wsa_hi = wsa.astype(BF)
    wsa_lo = (wsa - wsa_hi.astype(np.float32)).astype(BF)
    wcatb = np.concatenate(
        [w_mat.astype(BF), wsa_hi, wsa_lo], axis=1)  # [128, 144]

    in_maps = []
    for c in range(N_CORES):
        in_maps.append({
            "xTh": xTh,
            "xTl": xTl,
            "xlTh": np.ascontiguousarray(xTh[:, c * NPC:(c + 1) * NPC]),
            "xlTl": np.ascontiguousarray(xTl[:, c * NPC:(c + 1) * NPC]),
            "xloc": np.ascontiguousarray(xpad[c * NPC:(c + 1) * NPC]),
            "wcatb": wcatb,
            "bias": np.ascontiguousarray(bias, dtype=np.float32).reshape(1, NHF),
            "gidx": np.ascontiguousarray(gidx[c]),
            "rels": np.ascontiguousarray(rel_arr[c]),
            "selt": np.ascontiguousarray(selt[c]),
        })
    return t_a, t_b, in_maps


def kernel(x, edge_index, W, a_src, a_trg, bias, _trace=False):
    from concourse.bass_utils import run_bass_kernel_spmd

    x = np.asarray(x)
    t_a, t_b, in_maps = _make_inputs(x, np.asarray(edge_index),
                                     np.asarray(W, dtype=np.float32),
                                     np.asarray(a_src, dtype=np.float32),
                                     np.asarray(a_trg, dtype=np.float32),
                                     np.asarray(bias, dtype=np.float32))
    nc = build_bass(t_a, t_b)
    res = run_bass_kernel_spmd(nc, in_maps, core_ids=list(range(N_CORES)),
                               trace=_trace)
    parts = []
    for c in range(N_CORES):
        valid = min(NPC, N_NODES - c * NPC)
        parts.append(res.results[c]["out"][:valid])
    out = np.concatenate(parts, axis=0)
    if _trace:
        kernel.last_results = res
    return out.astype(np.float32)
